# revision 5
# baseline (speedup 1.0000x reference)
"""Trainium2 Bass kernel for nn_Decoder_48644799594904 (ConvS2S-style decoder).

Sharding: data-parallel over batch — B=8, one batch element per NeuronCore.

Per-core design (everything feature-major [feature, time]; no device
transposes anywhere):
  - conv layers = 3-tap accumulated matmuls over the fp32r state [H, T+2]
    (2 left-pad columns live inside the state buffer).
  - attention "layout B": energy is computed transposed [s_block, t], so the
    exp output p[s, t] is directly the moving operand of the PV matmul
    (contraction over s on partitions). Softmax denominators come from a
    ones-vector matmul on the PE; the reciprocal row is broadcast across
    partitions with a K=1 matmul and applied with one DVE multiply.
  - the graded attention output [t, s] is recomputed in layout A in a final
    phase with fp32 exp + ACT-accumulated row sums, which also provides the
    exact per-t reciprocals used to normalize attended_src.

Precision: fp32r (full PE rate at N>=256; ~16x more precise than bf16) for
conv/linear/energy matmuls, bf16 only for attention-probability matmuls.
All sqrt(0.5) residual scales are folded into host-side weight copies; the
state is stored pre-scaled by S^-layer so each residual update is just two
tensor_tensor adds and one tensor_scalar.
"""
import contextlib

import ml_dtypes
import numpy as np

import concourse.bass as bass  # noqa: F401
import concourse.mybir as mybir
import concourse.tile as tile
from concourse import bacc
from concourse import bass_utils

F32 = mybir.dt.float32
F32R = mybir.dt.float32r
BF16 = mybir.dt.bfloat16
AF = mybir.ActivationFunctionType
ALU = mybir.AluOpType

EMB, HID, NL, KSZ = 256, 512, 3, 3
B, T_FULL, S_FULL = 8, 2048, 2048
LB = -4294967295.0
SCALE = float(np.sqrt(0.5))


def _bf16(x):
    return np.ascontiguousarray(np.asarray(x).astype(ml_dtypes.bfloat16))


def _f32(x):
    return np.ascontiguousarray(np.asarray(x, dtype=np.float32))


class Dims:
    def __init__(self, T, S, Tc):
        assert T % Tc == 0 and Tc % 128 == 0 and S % 256 == 0
        self.T, self.S, self.Tc = T, S, Tc
        self.ET = EMB // 128           # e-tiles (2)
        self.HT = HID // 128           # h-tiles (4)
        self.SB = S // 128             # s-blocks
        self.NCH = T // Tc             # attention chunks
        self.TBC = Tc // 128           # t-blocks per chunk
        self.TB = T // 128             # t-blocks
        self.SH = S // 2               # final-phase energy half width
        self.CW = min(512, Tc)         # psum chunk width
        self.NCONV = T // self.CW      # conv chunks over T


# consts column map
def _cols(d):
    COL_BE2H = 0
    COL_BG = COL_BE2H + d.HT
    COL_BAH2E = COL_BG + NL * d.HT
    COL_BAE2H = COL_BAH2E + d.ET
    NCOLS = COL_BAE2H + d.HT
    return COL_BE2H, COL_BG, COL_BAH2E, COL_BAE2H, NCOLS


def build_program(d: Dims, has_mask: bool):
    nc = bacc.Bacc("TRN2", target_bir_lowering=False, debug=False)
    T, S, Tc = d.T, d.S, d.Tc
    ET, HT, SB, NCH, TBC, TB = d.ET, d.HT, d.SB, d.NCH, d.TBC, d.TB
    CW, NCONV, SH = d.CW, d.NCONV, d.SH
    COL_BE2H, COL_BG, COL_BAH2E, COL_BAE2H, NCOLS = _cols(d)
    coef_ae = [SCALE ** (1 - i) for i in range(NL)]
    mm = nc.tensor.matmul

    dram = {}

    def din(name, shape, dt):
        dram[name] = nc.dram_tensor(name, shape, dt, kind="ExternalInput").ap()
        return dram[name]

    din("embedded_fm", [EMB, T], F32R)
    din("enc_conved_fm", [EMB, S], F32R)
    din("enc_embed_b", [S, EMB], BF16)
    din("src_b", [S, EMB], BF16)
    din("convw", [NL, HT, 128, KSZ, HT, 256], F32R)
    din("w_e2h", [EMB, HID], F32R)
    din("w_ah2e", [NL, HID, EMB], F32R)
    din("w_ae2h", [EMB, HID], F32R)
    din("w_out_mov", [HID, EMB], F32R)
    din("consts", [128, NCOLS], F32)
    din("bias_rows", [1, NL * HID + EMB], F32R)
    din("padc", [NL, 128, HT, 2], F32R)
    din("ones_const", [1, 640], F32R)
    if has_mask:
        din("c_tile", [128, SB], F32)
        din("c_row", [1, S], F32R)

    out_d = nc.dram_tensor("out", [T, EMB], F32, kind="ExternalOutput").ap()
    att_d = nc.dram_tensor("att", [T, S], F32, kind="ExternalOutput").ap()

    with tile.TileContext(nc) as tc, contextlib.ExitStack() as ctx:
        sbm = ctx.enter_context(tc.tile_pool(name="sbm", bufs=1))
        wk1 = ctx.enter_context(tc.tile_pool(name="wk1", bufs=1))
        wk2 = ctx.enter_context(tc.tile_pool(name="wk2", bufs=2))

        # ---------- persistent SBUF ----------
        vstate = sbm.tile([128, HT, T + 2], F32R, name="vstate")
        ce_fm = sbm.tile([128, ET, T], F32R, name="ce_fm")  # also embedded buf
        encc = sbm.tile([128, ET, S], F32R, name="encc")
        ence = sbm.tile([128, SB, EMB], BF16, name="ence")
        srcb = sbm.tile([128, SB, EMB], BF16, name="srcb")
        ptile = sbm.tile([128, SB, Tc], BF16, name="ptile")
        we2h = sbm.tile([128, ET, HID], F32R, name="we2h")
        wae2h = sbm.tile([128, ET, HID], F32R, name="wae2h")
        wout = sbm.tile([128, HT, EMB], F32R, name="wout")
        consts = sbm.tile([128, NCOLS], F32, name="consts")
        brows = sbm.tile([1, NL * HID + EMB], F32R, name="brows")
        ones_r1 = sbm.tile([1, CW], F32R, name="ones_r1")
        ones128 = sbm.tile([1, 128], F32R, name="ones128")
        ones_bf = sbm.tile([128, 1], BF16, name="ones_bf")
        attsrc_sb = sbm.tile([128, TB, EMB], BF16, name="attsrc_sb")
        cmask = sbm.tile([128, SB], F32, name="cmask") if has_mask else None
        crow = sbm.tile([1, S], F32R, name="crow") if has_mask else None

        nc.vector.memset(ones_bf, 1.0)

        def dma(out, in_):
            nc.sync.dma_start(out=out, in_=in_)

        dma(ones_r1, dram["ones_const"][:, 0:CW])
        dma(ones128, dram["ones_const"][:, 0:128])
        dma(encc, dram["enc_conved_fm"].rearrange("(et p) s -> p et s", p=128))
        dma(ence, dram["enc_embed_b"].rearrange("(sb p) e -> p sb e", p=128))
        dma(srcb, dram["src_b"].rearrange("(sb p) e -> p sb e", p=128))
        dma(we2h, dram["w_e2h"].rearrange("(et p) h -> p et h", p=128))
        dma(wae2h, dram["w_ae2h"].rearrange("(et p) h -> p et h", p=128))
        dma(wout, dram["w_out_mov"].rearrange("(ht p) e -> p ht e", p=128))
        dma(consts, dram["consts"])
        dma(brows, dram["bias_rows"])
        if has_mask:
            dma(cmask, dram["c_tile"])
            dma(crow, dram["c_row"])
        dma(ce_fm, dram["embedded_fm"].rearrange("(et p) t -> p et t", p=128))

        with contextlib.ExitStack() as mainctx:
            ph1 = mainctx.enter_context(tc.tile_pool(name="ph1", bufs=1))
            psA = mainctx.enter_context(
                tc.tile_pool(name="psA", bufs=3, space="PSUM"))
            psG = mainctx.enter_context(
                tc.tile_pool(name="psG", bufs=2, space="PSUM"))
            psAcc = mainctx.enter_context(
                tc.tile_pool(name="psAcc", bufs=1, space="PSUM"))

            glu = ph1.tile([128, HT, T], F32R, name="glu")

            # ---- phase 0: vstate = tanh(W_e2h^T @ embedded + b) ----
            for ht in range(HT):
                for n in range(NCONV):
                    ps = psA.tile([128, CW], F32, tag="big",
                                  name=f"e2h_{ht}_{n}")
                    for ket in range(ET):
                        mm(ps, we2h[:, ket, ht * 128:(ht + 1) * 128],
                           ce_fm[:, ket, n * CW:(n + 1) * CW],
                           start=(ket == 0), stop=(ket == ET - 1))
                    nc.scalar.activation(
                        vstate[:, ht, 2 + n * CW: 2 + (n + 1) * CW], ps,
                        AF.Tanh,
                        bias=consts[:, COL_BE2H + ht: COL_BE2H + ht + 1])

            # ---- layers ----
            for li in range(NL):
                dma(vstate[:, :, 0:2], dram["padc"][li])

                # conv + GLU
                for mp in range(HT):
                    wtile = wk2.tile([128, KSZ, HT, 256], F32R, tag="convw",
                                     name=f"convw_{li}_{mp}")
                    dma(wtile, dram["convw"][li, mp])
                    for n in range(NCONV):
                        a_ps = psA.tile([128, CW], F32, tag="big",
                                        name=f"a_{li}_{mp}_{n}")
                        g_ps = psG.tile([128, CW], F32, tag="gg",
                                        name=f"g_{li}_{mp}_{n}")
                        mm(a_ps,
                           brows[0:1, li * HID + mp * 128:
                                 li * HID + (mp + 1) * 128],
                           ones_r1[0:1, 0:CW], start=True, stop=False)
                        for tap in range(KSZ):
                            for kt in range(HT):
                                last = (tap == KSZ - 1 and kt == HT - 1)
                                rhs = vstate[:, kt,
                                             tap + n * CW: tap + n * CW + CW]
                                mm(a_ps, wtile[:, tap, kt, 0:128], rhs,
                                   start=False, stop=last)
                                mm(g_ps, wtile[:, tap, kt, 128:256], rhs,
                                   start=(tap == 0 and kt == 0), stop=last)
                        sg = wk1.tile([128, CW], F32, tag="sg",
                                      name=f"sg_{li}_{mp}_{n}")
                        nc.scalar.activation(
                            sg, g_ps, AF.Sigmoid,
                            bias=consts[:, COL_BG + li * HT + mp:
                                        COL_BG + li * HT + mp + 1])
                        nc.vector.tensor_mul(
                            glu[:, mp, n * CW:(n + 1) * CW], a_ps, sg)

                # conved_emb
                wl = wk1.tile([128, HT, EMB], F32R, tag="wah2e",
                              name=f"wah2e_{li}")
                dma(wl, dram["w_ah2e"][li].rearrange(
                    "(ht p) e -> p ht e", p=128))
                for et in range(ET):
                    for n in range(NCONV):
                        ps = psA.tile([128, CW], F32, tag="big",
                                      name=f"ce_{li}_{et}_{n}")
                        for kt in range(HT):
                            mm(ps, wl[:, kt, et * 128:(et + 1) * 128],
                               glu[:, kt, n * CW:(n + 1) * CW],
                               start=(kt == 0), stop=(kt == HT - 1))
                        nc.scalar.activation(
                            ce_fm[:, et, n * CW:(n + 1) * CW], ps, AF.Tanh,
                            bias=consts[:, COL_BAH2E + et: COL_BAH2E + et + 1])

                # attention chunks
                for c in range(NCH):
                    t0 = c * Tc
                    attu = [psAcc.tile([128, Tc], F32, tag=f"attu{e}",
                                       name=f"attu{e}_{li}_{c}")
                            for e in range(ET)]
                    sums = psAcc.tile([1, Tc], F32, tag="sums",
                                      name=f"sums_{li}_{c}")
                    for sb in range(SB):
                        e_ps = psA.tile([128, Tc], F32, tag="big",
                                        name=f"en_{li}_{c}_{sb}")
                        for et in range(ET):
                            mm(e_ps, encc[:, et, sb * 128:(sb + 1) * 128],
                               ce_fm[:, et, t0:t0 + Tc],
                               start=(et == 0), stop=(et == ET - 1))
                        nc.scalar.activation(
                            ptile[:, sb, :], e_ps, AF.Exp,
                            bias=(cmask[:, sb:sb + 1] if has_mask else 0.0))
                        mm(sums, ones_bf, ptile[:, sb, :],
                           start=(sb == 0), stop=(sb == SB - 1))
                        for et in range(ET):
                            mm(attu[et],
                               ence[:, sb, et * 128:(et + 1) * 128],
                               ptile[:, sb, :],
                               start=(sb == 0), stop=(sb == SB - 1))

                    r_row = wk1.tile([1, Tc], F32R, tag="rrow",
                                     name=f"rrow_{li}_{c}")
                    with nc.allow_low_precision(reason="f32r recip row"):
                        nc.vector.reciprocal(r_row, sums)
                    rb_ps = psA.tile([128, Tc], F32, tag="big",
                                     name=f"rb_{li}_{c}")
                    mm(rb_ps, ones128, r_row, start=True, stop=True)
                    r_bc = wk1.tile([128, Tc], F32, tag="rbc",
                                    name=f"rbc_{li}_{c}")
                    nc.vector.tensor_copy(r_bc, rb_ps)
                    att_fm = wk1.tile([128, ET, Tc], F32R, tag="attfm",
                                      name=f"attfm_{li}_{c}")
                    for et in range(ET):
                        nc.vector.tensor_mul(att_fm[:, et, :], attu[et], r_bc)

                    # attended_src PV (final layer only), while p is live
                    if li == NL - 1:
                        for tb in range(TBC):
                            aps = psAcc.tile([128, EMB], F32, tag="sums",
                                             name=f"attsrc_{c}_{tb}")
                            for sb in range(SB):
                                mm(aps,
                                   ptile[:, sb, tb * 128:(tb + 1) * 128],
                                   srcb[:, sb, :],
                                   start=(sb == 0), stop=(sb == SB - 1))
                            nc.vector.tensor_copy(
                                attsrc_sb[:, c * TBC + tb, :], aps)

                    # att_enc + residual per h-tile
                    for ht in range(HT):
                        ps = psA.tile([128, Tc], F32, tag="big",
                                      name=f"aeps_{li}_{c}_{ht}")
                        for et in range(ET):
                            mm(ps, wae2h[:, et, ht * 128:(ht + 1) * 128],
                               att_fm[:, et, :],
                               start=(et == 0), stop=(et == ET - 1))
                        ae = wk2.tile([128, Tc], F32, tag="ae",
                                      name=f"ae_{li}_{c}_{ht}")
                        nc.scalar.activation(
                            ae, ps, AF.Tanh,
                            bias=consts[:, COL_BAE2H + ht:COL_BAE2H + ht + 1])
                        ae2 = wk1.tile([128, Tc], F32, tag="ae2",
                                       name=f"ae2_{li}_{c}_{ht}")
                        nc.vector.tensor_scalar_mul(ae2, ae, coef_ae[li])
                        tsum = wk1.tile([128, Tc], F32, tag="tsum",
                                        name=f"tsum_{li}_{c}_{ht}")
                        nc.vector.tensor_add(
                            tsum, glu[:, ht, t0:t0 + Tc],
                            vstate[:, ht, 2 + t0: 2 + t0 + Tc])
                        nc.vector.tensor_add(
                            vstate[:, ht, 2 + t0: 2 + t0 + Tc], tsum, ae2)

        # ---------- final phase ----------
        with contextlib.ExitStack() as finctx:
            ph2 = finctx.enter_context(tc.tile_pool(name="ph2", bufs=2))
            psB = finctx.enter_context(
                tc.tile_pool(name="psB", bufs=2, space="PSUM"))
            for tb in range(TB):
                tt0 = tb * 128
                sumA = ph2.tile([128, 2], F32, tag="sumA", name=f"sumA_{tb}")
                pa = []
                for h in range(2):
                    e_ps = psB.tile([128, SH], F32, tag="eA",
                                    name=f"eA_{tb}_{h}")
                    for s0 in range(0, SH, 512):
                        w = min(512, SH - s0)
                        sg0 = h * SH + s0
                        for et in range(ET):
                            mm(e_ps[:, s0:s0 + w],
                               ce_fm[:, et, tt0:tt0 + 128],
                               encc[:, et, sg0:sg0 + w],
                               start=(et == 0),
                               stop=(et == ET - 1 and not has_mask))
                        if has_mask:
                            mm(e_ps[:, s0:s0 + w], ones128,
                               crow[0:1, sg0:sg0 + w],
                               start=False, stop=True)
                    p_att = ph2.tile([128, SH], F32, tag="p_att",
                                     name=f"pA_{tb}_{h}")
                    nc.scalar.activation(p_att, e_ps, AF.Exp,
                                         accum_out=sumA[:, h:h + 1])
                    pa.append(p_att)
                stot = ph2.tile([128, 1], F32, tag="stot", name=f"stot_{tb}")
                nc.vector.tensor_add(stot, sumA[:, 0:1], sumA[:, 1:2])
                rA = ph2.tile([128, 1], F32, tag="rA", name=f"rA_{tb}")
                nc.vector.reciprocal(rA, stot)
                for h in range(2):
                    oa = ph2.tile([128, SH], F32, tag="oa",
                                  name=f"oa_{tb}_{h}")
                    nc.vector.tensor_scalar_mul(oa, pa[h], rA)
                    dma(att_d[tt0:tt0 + 128, h * SH:(h + 1) * SH], oa)

                o_ps = psB.tile([128, EMB], F32, tag="ops", name=f"ops_{tb}")
                mm(o_ps, ones128, brows[0:1, NL * HID: NL * HID + EMB],
                   start=True, stop=False)
                for kt in range(HT):
                    mm(o_ps, vstate[:, kt, 2 + tt0: 2 + tt0 + 128],
                       wout[:, kt, :], start=False, stop=(kt == HT - 1))
                tsrc = ph2.tile([128, EMB], F32, tag="tsrc", name=f"ts_{tb}")
                nc.vector.tensor_scalar(
                    out=tsrc, in0=attsrc_sb[:, tb, :], scalar1=rA,
                    scalar2=SCALE, op0=ALU.mult, op1=ALU.mult)
                ofm = ph2.tile([128, EMB], F32, tag="ofm", name=f"ofm_{tb}")
                nc.vector.tensor_add(ofm, o_ps, tsrc)
                dma(out_d[tt0:tt0 + 128, :], ofm)

    nc.compile()
    return nc


# ----------------------------------------------------------------------------
# host side
# ----------------------------------------------------------------------------
_PROG_CACHE = {}


def _get_program(d: Dims, has_mask: bool):
    key = (d.T, d.S, d.Tc, has_mask)
    if key not in _PROG_CACHE:
        _PROG_CACHE[key] = build_program(d, has_mask)
    return _PROG_CACHE[key]


def prep_shared(inputs, d: Dims):
    """Host-side weight prep shared across cores (with scale folding)."""
    W_e2h = _f32(inputs["W_e2h"]); b_e2h = _f32(inputs["b_e2h"])
    W_ah2e = _f32(inputs["W_ah2e"]); b_ah2e = _f32(inputs["b_ah2e"])
    W_ae2h = _f32(inputs["W_ae2h"]); b_ae2h = _f32(inputs["b_ae2h"])
    W_out = _f32(inputs["W_out"]); b_out = _f32(inputs["b_out"])
    conv_w = _f32(inputs["conv_w"]); conv_b = _f32(inputs["conv_b"])
    padv = _f32(inputs["pad_vector"])[0, 0]
    HT, ET = d.HT, d.ET
    COL_BE2H, COL_BG, COL_BAH2E, COL_BAE2H, NCOLS = _cols(d)

    sh = {}
    sh["w_e2h"] = W_e2h
    # beta_i = S^(1-i): W_ah2e'^(i) = W_ah2e / beta_i
    sh["w_ah2e"] = np.ascontiguousarray(np.stack(
        [W_ah2e / (SCALE ** (1 - i)) for i in range(NL)]))
    sh["w_ae2h"] = W_ae2h
    sh["w_out_mov"] = _f32(W_out * (SCALE ** 4))

    # conv weights: dev[l, mp, p, tap, kt, o256]
    # W_a'^(i) = S * W_a ; W_g'^(i) = S^i * W_g  (state alpha_i = S^-i)
    wt = np.transpose(conv_w, (0, 2, 3, 1))  # [l, H, K, 2H]
    dev = np.empty((NL, HT, 128, KSZ, HT, 256), np.float32)
    for l in range(NL):
        a_sc = SCALE
        g_sc = SCALE ** l
        for mp in range(HT):
            for kt in range(HT):
                blk = wt[l, kt * 128:(kt + 1) * 128]  # [128, K, 2H]
                dev[l, mp, :, :, kt, 0:128] = (
                    blk[:, :, mp * 128:(mp + 1) * 128] * a_sc)
                dev[l, mp, :, :, kt, 128:256] = (
                    blk[:, :, HID + mp * 128: HID + (mp + 1) * 128] * g_sc)
    sh["convw"] = np.ascontiguousarray(dev)

    consts = np.zeros((128, NCOLS), np.float32)
    for h in range(HT):
        consts[:, COL_BE2H + h] = b_e2h[h * 128:(h + 1) * 128]
    for l in range(NL):
        for h in range(HT):
            consts[:, COL_BG + l * HT + h] = conv_b[l][HID + h * 128:
                                                       HID + (h + 1) * 128]
    for e in range(ET):
        consts[:, COL_BAH2E + e] = b_ah2e[e * 128:(e + 1) * 128]
    for h in range(HT):
        consts[:, COL_BAE2H + h] = b_ae2h[h * 128:(h + 1) * 128]
    sh["consts"] = consts

    br = np.zeros((1, NL * HID + EMB), np.float32)
    for l in range(NL):
        br[0, l * HID:(l + 1) * HID] = conv_b[l][:HID] * (SCALE ** (1 - l))
    br[0, NL * HID:] = b_out * SCALE
    sh["bias_rows"] = br

    pad_proj = np.tanh(padv @ W_e2h + b_e2h)  # [H]
    padc = np.empty((NL, 128, HT, 2), np.float32)
    for l in range(NL):
        for h in range(HT):
            v = pad_proj[h * 128:(h + 1) * 128] * (SCALE ** (-l))
            padc[l, :, h, 0] = v
            padc[l, :, h, 1] = v
    sh["padc"] = padc
    return sh


def make_in_maps(inputs, d: Dims, nb: int):
    trg = _f32(inputs["trg"]); src = _f32(inputs["src"])
    encc = _f32(inputs["encoder_conved"]); ence = _f32(inputs["encoder_embed"])
    mask = _f32(inputs["mask"]); pos = _f32(inputs["pos_emb"])
    sh = prep_shared(inputs, d)
    cvals = (1.0 - mask[:, 0, :]) * LB
    has_mask = bool(np.any(cvals != 0.0))
    in_maps = []
    for b in range(nb):
        m = {
            "embedded_fm": np.ascontiguousarray((trg[b] + pos[:d.T]).T),
            "enc_conved_fm": np.ascontiguousarray(encc[b].T),
            "enc_embed_b": _bf16(ence[b]),
            "src_b": _bf16(src[b]),
            "convw": sh["convw"],
            "w_e2h": sh["w_e2h"],
            "w_ah2e": sh["w_ah2e"],
            "w_ae2h": sh["w_ae2h"],
            "w_out_mov": sh["w_out_mov"],
            "consts": sh["consts"],
            "bias_rows": sh["bias_rows"],
            "padc": sh["padc"],
            "ones_const": np.ones((1, 640), np.float32),
        }
        if has_mask:
            ct = np.zeros((128, d.SB), np.float32)
            for sb in range(d.SB):
                ct[:, sb] = cvals[b, sb * 128:(sb + 1) * 128]
            m["c_tile"] = ct
            m["c_row"] = np.ascontiguousarray(cvals[b][None, :])
        in_maps.append(m)
    return in_maps, has_mask


def kernel(**inputs):
    d = Dims(T_FULL, S_FULL, 512)
    in_maps, has_mask = make_in_maps(inputs, d, B)
    nc = _get_program(d, has_mask)
    res = bass_utils.run_bass_kernel_spmd(nc, in_maps, core_ids=list(range(B)))
    output = np.stack([res.results[b]["out"] for b in range(B)])
    attention = np.stack([res.results[b]["att"] for b in range(B)])
    return output, attention


# revision 8
# speedup vs baseline: 1.1331x; 1.1331x over previous
"""Trainium2 Bass kernel for nn_Decoder_48644799594904 (ConvS2S-style decoder).

Sharding: data-parallel over batch — B=8, one batch element per NeuronCore.

Per-core design (everything feature-major [feature, time]; no device
transposes anywhere):
  - conv layers = 3-tap accumulated matmuls over the fp32r state [H, T+2]
    (2 left-pad columns live inside the state buffer).
  - attention "layout B": energy is computed transposed [s_block, t], so the
    exp output p[s, t] is directly the moving operand of the PV matmul
    (contraction over s on partitions). Softmax denominators come from a
    ones-vector matmul on the PE; the reciprocal row is broadcast across
    partitions with a K=1 matmul and applied with one DVE multiply.
  - the graded attention output [t, s] is recomputed in layout A in a final
    phase with fp32 exp + ACT-accumulated row sums, which also provides the
    exact per-t reciprocals used to normalize attended_src.

Precision: fp32r (full PE rate at N>=256; ~16x more precise than bf16) for
conv/linear/energy matmuls, bf16 only for attention-probability matmuls.
All sqrt(0.5) residual scales are folded into host-side weight copies; the
state is stored pre-scaled by S^-layer so each residual update is just two
tensor_tensor adds and one tensor_scalar.
"""
import contextlib

import ml_dtypes
import numpy as np

import concourse.bass as bass  # noqa: F401
import concourse.mybir as mybir
import concourse.tile as tile
from concourse import bacc
from concourse import bass_utils

F32 = mybir.dt.float32
F32R = mybir.dt.float32r
BF16 = mybir.dt.bfloat16
AF = mybir.ActivationFunctionType
ALU = mybir.AluOpType

EMB, HID, NL, KSZ = 256, 512, 3, 3
B, T_FULL, S_FULL = 8, 2048, 2048
LB = -4294967295.0
SCALE = float(np.sqrt(0.5))


def _bf16(x):
    return np.ascontiguousarray(np.asarray(x).astype(ml_dtypes.bfloat16))


def _f32(x):
    return np.ascontiguousarray(np.asarray(x, dtype=np.float32))


class Dims:
    def __init__(self, T, S, Tc):
        assert T % Tc == 0 and Tc % 128 == 0 and S % 256 == 0
        self.T, self.S, self.Tc = T, S, Tc
        self.ET = EMB // 128           # e-tiles (2)
        self.HT = HID // 128           # h-tiles (4)
        self.SB = S // 128             # s-blocks
        self.NCH = T // Tc             # attention chunks
        self.TBC = Tc // 128           # t-blocks per chunk
        self.TB = T // 128             # t-blocks
        self.SH = S // 2               # final-phase energy half width
        self.CW = min(512, Tc)         # psum chunk width
        self.NCONV = T // self.CW      # conv chunks over T


# consts column map
def _cols(d):
    COL_BE2H = 0
    COL_BG = COL_BE2H + d.HT
    COL_BAH2E = COL_BG + NL * d.HT
    COL_BAE2H = COL_BAH2E + d.ET
    COL_BA = COL_BAE2H + d.HT
    NCOLS = COL_BA + NL * d.HT
    return COL_BE2H, COL_BG, COL_BAH2E, COL_BAE2H, COL_BA, NCOLS


def build_program(d: Dims, has_mask: bool):
    nc = bacc.Bacc("TRN2", target_bir_lowering=False, debug=False)
    T, S, Tc = d.T, d.S, d.Tc
    ET, HT, SB, NCH, TBC, TB = d.ET, d.HT, d.SB, d.NCH, d.TBC, d.TB
    CW, NCONV, SH = d.CW, d.NCONV, d.SH
    COL_BE2H, COL_BG, COL_BAH2E, COL_BAE2H, COL_BA, NCOLS = _cols(d)
    coef_ae = [SCALE ** (1 - i) for i in range(NL)]
    mm = nc.tensor.matmul

    dram = {}

    def din(name, shape, dt):
        dram[name] = nc.dram_tensor(name, shape, dt, kind="ExternalInput").ap()
        return dram[name]

    din("embedded_fm", [EMB, T], F32R)
    din("enc_conved_fm", [EMB, S], F32R)
    din("enc_embed_b", [S, EMB], BF16)
    din("src_b", [S, EMB], BF16)
    din("convw", [NL, HT, 128, KSZ, HT, 256], F32R)
    din("w_e2h", [EMB, HID], F32R)
    din("w_ah2e", [NL, HID, EMB], F32R)
    din("w_ae2h", [EMB, HID], F32R)
    din("w_out_mov", [HID, EMB], F32R)
    din("consts", [128, NCOLS], F32)
    din("bias_rows", [1, NL * HID + EMB], F32R)
    din("padc", [NL, 128, HT, 2], F32R)
    din("ones_const", [1, 640], F32R)
    if has_mask:
        din("c_tile", [128, SB], F32)
        din("c_row", [1, S], F32R)

    out_d = nc.dram_tensor("out", [T, EMB], F32, kind="ExternalOutput").ap()
    att_d = nc.dram_tensor("att", [T, S], F32, kind="ExternalOutput").ap()

    with tile.TileContext(nc) as tc, contextlib.ExitStack() as ctx:
        sbm = ctx.enter_context(tc.tile_pool(name="sbm", bufs=1))
        wk1 = ctx.enter_context(tc.tile_pool(name="wk1", bufs=1))
        wk2 = ctx.enter_context(tc.tile_pool(name="wk2", bufs=2))

        # ---------- persistent SBUF ----------
        vstate = sbm.tile([128, HT, T + 2], F32R, name="vstate")
        ce_fm = sbm.tile([128, ET, T], F32R, name="ce_fm")  # also embedded buf
        encc = sbm.tile([128, ET, S], F32R, name="encc")
        ence = sbm.tile([128, SB, EMB], BF16, name="ence")
        srcb = sbm.tile([128, SB, EMB], BF16, name="srcb")
        ptile = sbm.tile([128, SB, Tc], BF16, name="ptile")
        we2h = sbm.tile([128, ET, HID], F32R, name="we2h")
        wae2h = sbm.tile([128, ET, HID], F32R, name="wae2h")
        wout = sbm.tile([128, HT, EMB], F32R, name="wout")
        consts = sbm.tile([128, NCOLS], F32, name="consts")
        brows = sbm.tile([1, NL * HID + EMB], F32R, name="brows")
        ones128 = sbm.tile([1, 128], F32R, name="ones128")
        ones_bf = sbm.tile([128, 1], BF16, name="ones_bf")
        attsrc_sb = sbm.tile([128, TB, EMB], BF16, name="attsrc_sb")
        cmask = sbm.tile([128, SB], F32, name="cmask") if has_mask else None
        crow = sbm.tile([1, S], F32R, name="crow") if has_mask else None

        nc.vector.memset(ones_bf, 1.0)

        def dma(out, in_):
            nc.sync.dma_start(out=out, in_=in_)

        dma(ones128, dram["ones_const"][:, 0:128])
        dma(encc, dram["enc_conved_fm"].rearrange("(et p) s -> p et s", p=128))
        dma(ence, dram["enc_embed_b"].rearrange("(sb p) e -> p sb e", p=128))
        dma(srcb, dram["src_b"].rearrange("(sb p) e -> p sb e", p=128))
        dma(we2h, dram["w_e2h"].rearrange("(et p) h -> p et h", p=128))
        dma(wae2h, dram["w_ae2h"].rearrange("(et p) h -> p et h", p=128))
        dma(wout, dram["w_out_mov"].rearrange("(ht p) e -> p ht e", p=128))
        dma(consts, dram["consts"])
        dma(brows, dram["bias_rows"])
        if has_mask:
            dma(cmask, dram["c_tile"])
            dma(crow, dram["c_row"])
        dma(ce_fm, dram["embedded_fm"].rearrange("(et p) t -> p et t", p=128))

        with contextlib.ExitStack() as mainctx:
            ph1 = mainctx.enter_context(tc.tile_pool(name="ph1", bufs=1))
            psA = mainctx.enter_context(
                tc.tile_pool(name="psA", bufs=3, space="PSUM"))
            psG = mainctx.enter_context(
                tc.tile_pool(name="psG", bufs=2, space="PSUM"))
            psAcc = mainctx.enter_context(
                tc.tile_pool(name="psAcc", bufs=1, space="PSUM"))

            glu = ph1.tile([128, HT, T], F32R, name="glu")

            # ---- phase 0: vstate = tanh(W_e2h^T @ embedded + b) ----
            for ht in range(HT):
                for n in range(NCONV):
                    ps = psA.tile([128, CW], F32, tag="big",
                                  name=f"e2h_{ht}_{n}")
                    for ket in range(ET):
                        mm(ps, we2h[:, ket, ht * 128:(ht + 1) * 128],
                           ce_fm[:, ket, n * CW:(n + 1) * CW],
                           start=(ket == 0), stop=(ket == ET - 1))
                    nc.scalar.activation(
                        vstate[:, ht, 2 + n * CW: 2 + (n + 1) * CW], ps,
                        AF.Tanh,
                        bias=consts[:, COL_BE2H + ht: COL_BE2H + ht + 1])

            # ---- layers ----
            for li in range(NL):
                dma(vstate[:, :, 0:2], dram["padc"][li])

                # conv + GLU
                for mp in range(HT):
                    wtile = wk2.tile([128, KSZ, HT, 256], F32R, tag="convw",
                                     name=f"convw_{li}_{mp}")
                    dma(wtile, dram["convw"][li, mp])
                    for n in range(NCONV):
                        a_ps = psA.tile([128, CW], F32, tag="big",
                                        name=f"a_{li}_{mp}_{n}")
                        g_ps = psG.tile([128, CW], F32, tag="gg",
                                        name=f"g_{li}_{mp}_{n}")
                        for tap in range(KSZ):
                            for kt in range(HT):
                                last = (tap == KSZ - 1 and kt == HT - 1)
                                rhs = vstate[:, kt,
                                             tap + n * CW: tap + n * CW + CW]
                                mm(a_ps, wtile[:, tap, kt, 0:128], rhs,
                                   start=(tap == 0 and kt == 0), stop=last)
                                mm(g_ps, wtile[:, tap, kt, 128:256], rhs,
                                   start=(tap == 0 and kt == 0), stop=last)
                        sg = wk1.tile([128, CW], F32, tag="sg",
                                      name=f"sg_{li}_{mp}_{n}")
                        nc.scalar.activation(
                            sg, g_ps, AF.Sigmoid,
                            bias=consts[:, COL_BG + li * HT + mp:
                                        COL_BG + li * HT + mp + 1])
                        ab = wk1.tile([128, CW], F32, tag="tsum",
                                      name=f"ab_{li}_{mp}_{n}")
                        nc.vector.tensor_scalar_add(
                            ab, a_ps, consts[:, COL_BA + li * HT + mp:
                                             COL_BA + li * HT + mp + 1])
                        nc.vector.tensor_mul(
                            glu[:, mp, n * CW:(n + 1) * CW], ab, sg)

                # conved_emb
                wl = wk1.tile([128, HT, EMB], F32R, tag="wah2e",
                              name=f"wah2e_{li}")
                dma(wl, dram["w_ah2e"][li].rearrange(
                    "(ht p) e -> p ht e", p=128))
                for et in range(ET):
                    for n in range(NCONV):
                        ps = psA.tile([128, CW], F32, tag="big",
                                      name=f"ce_{li}_{et}_{n}")
                        for kt in range(HT):
                            mm(ps, wl[:, kt, et * 128:(et + 1) * 128],
                               glu[:, kt, n * CW:(n + 1) * CW],
                               start=(kt == 0), stop=(kt == HT - 1))
                        nc.scalar.activation(
                            ce_fm[:, et, n * CW:(n + 1) * CW], ps, AF.Tanh,
                            bias=consts[:, COL_BAH2E + et: COL_BAH2E + et + 1])

                # attention chunks
                for c in range(NCH):
                    t0 = c * Tc
                    attu = [psAcc.tile([128, Tc], F32, tag=f"attu{e}",
                                       name=f"attu{e}_{li}_{c}")
                            for e in range(ET)]
                    sums = psAcc.tile([1, Tc], F32, tag="sums",
                                      name=f"sums_{li}_{c}")
                    for sb in range(SB):
                        e_ps = psA.tile([128, Tc], F32, tag="big",
                                        name=f"en_{li}_{c}_{sb}")
                        for et in range(ET):
                            mm(e_ps, encc[:, et, sb * 128:(sb + 1) * 128],
                               ce_fm[:, et, t0:t0 + Tc],
                               start=(et == 0), stop=(et == ET - 1))
                        nc.scalar.activation(
                            ptile[:, sb, :], e_ps, AF.Exp,
                            bias=(cmask[:, sb:sb + 1] if has_mask else 0.0))
                        for et in range(ET):
                            mm(attu[et],
                               ence[:, sb, et * 128:(et + 1) * 128],
                               ptile[:, sb, :],
                               start=(sb == 0), stop=(sb == SB - 1))

                    for sb in range(SB):
                        mm(sums, ones_bf, ptile[:, sb, :],
                           start=(sb == 0), stop=(sb == SB - 1))
                    r_tmp = wk1.tile([1, Tc], F32, tag="rtmp",
                                     name=f"rtmp_{li}_{c}")
                    nc.vector.tensor_copy(r_tmp, sums)
                    r_fast = wk1.tile([1, Tc], F32, tag="rfast",
                                      name=f"rfast_{li}_{c}")
                    nc.vector.reciprocal_approx_fast(out=r_fast, in_=r_tmp)
                    r_row = wk1.tile([1, Tc], F32R, tag="rrow",
                                     name=f"rrow_{li}_{c}")
                    nc.vector.tensor_copy(r_row, r_fast)
                    rb_ps = psA.tile([128, Tc], F32, tag="big",
                                     name=f"rb_{li}_{c}")
                    mm(rb_ps, ones128, r_row, start=True, stop=True)
                    r_bc = wk1.tile([128, Tc], F32, tag="rbc",
                                    name=f"rbc_{li}_{c}")
                    nc.vector.tensor_copy(r_bc, rb_ps)
                    att_fm = wk1.tile([128, ET, Tc], F32R, tag="attfm",
                                      name=f"attfm_{li}_{c}")
                    for et in range(ET):
                        nc.vector.tensor_mul(att_fm[:, et, :], attu[et], r_bc)

                    # attended_src PV (final layer only), while p is live
                    if li == NL - 1:
                        for tb in range(TBC):
                            aps = psAcc.tile([128, EMB], F32, tag="sums",
                                             name=f"attsrc_{c}_{tb}")
                            for sb in range(SB):
                                mm(aps,
                                   ptile[:, sb, tb * 128:(tb + 1) * 128],
                                   srcb[:, sb, :],
                                   start=(sb == 0), stop=(sb == SB - 1))
                            nc.vector.tensor_copy(
                                attsrc_sb[:, c * TBC + tb, :], aps)

                    # att_enc + residual per h-tile
                    for ht in range(HT):
                        ps = psA.tile([128, Tc], F32, tag="big",
                                      name=f"aeps_{li}_{c}_{ht}")
                        for et in range(ET):
                            mm(ps, wae2h[:, et, ht * 128:(ht + 1) * 128],
                               att_fm[:, et, :],
                               start=(et == 0), stop=(et == ET - 1))
                        ae = wk2.tile([128, Tc], F32, tag="ae",
                                      name=f"ae_{li}_{c}_{ht}")
                        nc.scalar.activation(
                            ae, ps, AF.Tanh,
                            bias=consts[:, COL_BAE2H + ht:COL_BAE2H + ht + 1])
                        ae2 = wk1.tile([128, Tc], F32, tag="ae2",
                                       name=f"ae2_{li}_{c}_{ht}")
                        nc.vector.tensor_scalar_mul(ae2, ae, coef_ae[li])
                        tsum = wk1.tile([128, Tc], F32, tag="tsum",
                                        name=f"tsum_{li}_{c}_{ht}")
                        nc.vector.tensor_add(
                            tsum, glu[:, ht, t0:t0 + Tc],
                            vstate[:, ht, 2 + t0: 2 + t0 + Tc])
                        nc.vector.tensor_add(
                            vstate[:, ht, 2 + t0: 2 + t0 + Tc], tsum, ae2)

        # ---------- final phase ----------
        with contextlib.ExitStack() as finctx:
            ph2 = finctx.enter_context(tc.tile_pool(name="ph2", bufs=2))
            psB = finctx.enter_context(
                tc.tile_pool(name="psB", bufs=2, space="PSUM"))
            for tb in range(TB):
                tt0 = tb * 128
                sumA = ph2.tile([128, 2], F32, tag="sumA", name=f"sumA_{tb}")
                pa = []
                for h in range(2):
                    e_ps = psB.tile([128, SH], F32, tag="eA",
                                    name=f"eA_{tb}_{h}")
                    for s0 in range(0, SH, 512):
                        w = min(512, SH - s0)
                        sg0 = h * SH + s0
                        for et in range(ET):
                            mm(e_ps[:, s0:s0 + w],
                               ce_fm[:, et, tt0:tt0 + 128],
                               encc[:, et, sg0:sg0 + w],
                               start=(et == 0),
                               stop=(et == ET - 1 and not has_mask))
                        if has_mask:
                            mm(e_ps[:, s0:s0 + w], ones128,
                               crow[0:1, sg0:sg0 + w],
                               start=False, stop=True)
                    p_att = ph2.tile([128, SH], F32, tag="p_att",
                                     name=f"pA_{tb}_{h}")
                    nc.scalar.activation(p_att, e_ps, AF.Exp,
                                         accum_out=sumA[:, h:h + 1])
                    pa.append(p_att)
                stot = ph2.tile([128, 1], F32, tag="stot", name=f"stot_{tb}")
                nc.vector.tensor_add(stot, sumA[:, 0:1], sumA[:, 1:2])
                rA = ph2.tile([128, 1], F32, tag="rA", name=f"rA_{tb}")
                nc.vector.reciprocal(rA, stot)
                for h in range(2):
                    oa = ph2.tile([128, SH], F32, tag="oa",
                                  name=f"oa_{tb}_{h}")
                    nc.vector.tensor_scalar_mul(oa, pa[h], rA)
                    dma(att_d[tt0:tt0 + 128, h * SH:(h + 1) * SH], oa)

                o_ps = psB.tile([128, EMB], F32, tag="ops", name=f"ops_{tb}")
                mm(o_ps, ones128, brows[0:1, NL * HID: NL * HID + EMB],
                   start=True, stop=False)
                for kt in range(HT):
                    mm(o_ps, vstate[:, kt, 2 + tt0: 2 + tt0 + 128],
                       wout[:, kt, :], start=False, stop=(kt == HT - 1))
                tsrc = ph2.tile([128, EMB], F32, tag="tsrc", name=f"ts_{tb}")
                nc.vector.tensor_scalar(
                    out=tsrc, in0=attsrc_sb[:, tb, :], scalar1=rA,
                    scalar2=SCALE, op0=ALU.mult, op1=ALU.mult)
                ofm = ph2.tile([128, EMB], F32, tag="ofm", name=f"ofm_{tb}")
                nc.vector.tensor_add(ofm, o_ps, tsrc)
                dma(out_d[tt0:tt0 + 128, :], ofm)

    nc.compile()
    return nc


# ----------------------------------------------------------------------------
# host side
# ----------------------------------------------------------------------------
_PROG_CACHE = {}


def _get_program(d: Dims, has_mask: bool):
    key = (d.T, d.S, d.Tc, has_mask)
    if key not in _PROG_CACHE:
        _PROG_CACHE[key] = build_program(d, has_mask)
    return _PROG_CACHE[key]


def prep_shared(inputs, d: Dims):
    """Host-side weight prep shared across cores (with scale folding)."""
    W_e2h = _f32(inputs["W_e2h"]); b_e2h = _f32(inputs["b_e2h"])
    W_ah2e = _f32(inputs["W_ah2e"]); b_ah2e = _f32(inputs["b_ah2e"])
    W_ae2h = _f32(inputs["W_ae2h"]); b_ae2h = _f32(inputs["b_ae2h"])
    W_out = _f32(inputs["W_out"]); b_out = _f32(inputs["b_out"])
    conv_w = _f32(inputs["conv_w"]); conv_b = _f32(inputs["conv_b"])
    padv = _f32(inputs["pad_vector"])[0, 0]
    HT, ET = d.HT, d.ET
    COL_BE2H, COL_BG, COL_BAH2E, COL_BAE2H, COL_BA, NCOLS = _cols(d)

    sh = {}
    sh["w_e2h"] = W_e2h
    # beta_i = S^(1-i): W_ah2e'^(i) = W_ah2e / beta_i
    sh["w_ah2e"] = np.ascontiguousarray(np.stack(
        [W_ah2e / (SCALE ** (1 - i)) for i in range(NL)]))
    sh["w_ae2h"] = W_ae2h
    sh["w_out_mov"] = _f32(W_out * (SCALE ** 4))

    # conv weights: dev[l, mp, p, tap, kt, o256]
    # W_a'^(i) = S * W_a ; W_g'^(i) = S^i * W_g  (state alpha_i = S^-i)
    wt = np.transpose(conv_w, (0, 2, 3, 1))  # [l, H, K, 2H]
    dev = np.empty((NL, HT, 128, KSZ, HT, 256), np.float32)
    for l in range(NL):
        a_sc = SCALE
        g_sc = SCALE ** l
        for mp in range(HT):
            for kt in range(HT):
                blk = wt[l, kt * 128:(kt + 1) * 128]  # [128, K, 2H]
                dev[l, mp, :, :, kt, 0:128] = (
                    blk[:, :, mp * 128:(mp + 1) * 128] * a_sc)
                dev[l, mp, :, :, kt, 128:256] = (
                    blk[:, :, HID + mp * 128: HID + (mp + 1) * 128] * g_sc)
    sh["convw"] = np.ascontiguousarray(dev)

    consts = np.zeros((128, NCOLS), np.float32)
    for h in range(HT):
        consts[:, COL_BE2H + h] = b_e2h[h * 128:(h + 1) * 128]
    for l in range(NL):
        for h in range(HT):
            consts[:, COL_BG + l * HT + h] = conv_b[l][HID + h * 128:
                                                       HID + (h + 1) * 128]
    for e in range(ET):
        consts[:, COL_BAH2E + e] = b_ah2e[e * 128:(e + 1) * 128]
    for h in range(HT):
        consts[:, COL_BAE2H + h] = b_ae2h[h * 128:(h + 1) * 128]
    for l in range(NL):
        beta = SCALE ** (1 - l)
        for h in range(HT):
            consts[:, COL_BA + l * HT + h] = (
                conv_b[l][h * 128:(h + 1) * 128] * beta)
    sh["consts"] = consts

    br = np.zeros((1, NL * HID + EMB), np.float32)
    for l in range(NL):
        br[0, l * HID:(l + 1) * HID] = conv_b[l][:HID] * (SCALE ** (1 - l))
    br[0, NL * HID:] = b_out * SCALE
    sh["bias_rows"] = br

    pad_proj = np.tanh(padv @ W_e2h + b_e2h)  # [H]
    padc = np.empty((NL, 128, HT, 2), np.float32)
    for l in range(NL):
        for h in range(HT):
            v = pad_proj[h * 128:(h + 1) * 128] * (SCALE ** (-l))
            padc[l, :, h, 0] = v
            padc[l, :, h, 1] = v
    sh["padc"] = padc
    return sh


def make_in_maps(inputs, d: Dims, nb: int):
    trg = _f32(inputs["trg"]); src = _f32(inputs["src"])
    encc = _f32(inputs["encoder_conved"]); ence = _f32(inputs["encoder_embed"])
    mask = _f32(inputs["mask"]); pos = _f32(inputs["pos_emb"])
    sh = prep_shared(inputs, d)
    cvals = (1.0 - mask[:, 0, :]) * LB
    has_mask = bool(np.any(cvals != 0.0))
    in_maps = []
    for b in range(nb):
        m = {
            "embedded_fm": np.ascontiguousarray((trg[b] + pos[:d.T]).T),
            "enc_conved_fm": np.ascontiguousarray(encc[b].T),
            "enc_embed_b": _bf16(ence[b]),
            "src_b": _bf16(src[b]),
            "convw": sh["convw"],
            "w_e2h": sh["w_e2h"],
            "w_ah2e": sh["w_ah2e"],
            "w_ae2h": sh["w_ae2h"],
            "w_out_mov": sh["w_out_mov"],
            "consts": sh["consts"],
            "bias_rows": sh["bias_rows"],
            "padc": sh["padc"],
            "ones_const": np.ones((1, 640), np.float32),
        }
        if has_mask:
            ct = np.zeros((128, d.SB), np.float32)
            for sb in range(d.SB):
                ct[:, sb] = cvals[b, sb * 128:(sb + 1) * 128]
            m["c_tile"] = ct
            m["c_row"] = np.ascontiguousarray(cvals[b][None, :])
        in_maps.append(m)
    return in_maps, has_mask


def kernel(**inputs):
    d = Dims(T_FULL, S_FULL, 512)
    in_maps, has_mask = make_in_maps(inputs, d, B)
    nc = _get_program(d, has_mask)
    res = bass_utils.run_bass_kernel_spmd(nc, in_maps, core_ids=list(range(B)))
    output = np.stack([res.results[b]["out"] for b in range(B)])
    attention = np.stack([res.results[b]["att"] for b in range(B)])
    return output, attention
